# revision 9
# baseline (speedup 1.0000x reference)
"""Multi-head causal attention (B=8, S=2048, E=512, H=8, D=64) on 8 trn2 cores.

Strategy: pure data parallelism over the batch dimension (B == n_cores == 8).
Each NeuronCore computes the full attention for one batch element; no
collectives are needed.

v2 (vs baseline): attention matmul operands in bf16 (1 cyc/row on PE at any
width; fp32r pays 4x under 256 cols), x loaded in 4 batched DMAs instead of
16 (DMA-issue on the sync queue costs ~626ns each), qkT emission interleaved
with stripe-0 attention so ScalarE's exp stream starts early and the PE never
idles long enough for the HAM clock gate to re-throttle to 1.2 GHz, diagonal
causal mask applied as a DVE multiply with a precomputed 0/1 bf16 mask
(instead of a PE mask-matmul), softmax normalization via
reciprocal_approx_fast + PE ones-broadcast (instead of full-precision DVE
reciprocal at ~6.5us and a DRAM round-trip), PSUM->SBUF staging copy and
proj bias-add on GpSimd (idle otherwise), and the exp activation table
pre-warmed at kernel start.

Per core:
  1. Load x [S,E] (4 batched DMAs), transpose on PE -> xT bf16 [E,S].
  2. QKV: qT/kT per 2-head group g from wq/wk (bf16), v stored interleaved
     as vt[p=sk, h, j, 0:64] bf16 with vt[..., 64] = 1.0 so the AV matmul's
     stationary [128, 65] also produces the softmax denominator (row 64).
  3. Per sq-stripe (1024) and head: scoresT[sk,sq] = kT_j^T @ qT (K=64,
     bf16), exp via ScalarE (scale=1/8, no max subtraction needed:
     |scores/8| <~ 2) -> sb bf16, diag block masked by multiplying with a
     strictly-lower-zero mask on DVE, outT_aug[65,sq] += vt_j^T @ attT_j.
     Software-pipelined two deep so the PE does not stall on exp.
  4. Normalize: pst = po staged to SBUF (GpSimd, frees the PSUM bank),
     r = 1/l via reciprocal_approx_fast (DVE), broadcast r across 64
     partitions with a PE ones-matmul, attoutT = pst * r_bcast (DVE, bf16).
  5. Output projection out[s,e] = attoutT^T @ Wp (bf16) + bp (GpSimd add)
     -> HBM, interleaved with the next stripe's attention.

A single PSUM pool with two tags (4+4 banks) is used for the whole kernel.
Post-scheduling, excess semaphore waits are hoisted onto same-engine NoOps
(several ISA structs accept only one wait slot; walrus rejects multi-wait
instructions).
"""

import numpy as np
from contextlib import ExitStack

import concourse.bass as bass
import concourse.mybir as mybir
from concourse.tile import TileContext
from concourse.masks import make_identity
from concourse import bass_utils

F32 = mybir.dt.float32
F32R = mybir.dt.float32r
BF16 = mybir.dt.bfloat16
B, S_FULL, E, H, D = 8, 2048, 512, 8, 64
P = 128
G = H // 2      # 2-head groups
EB = E // P     # e blocks
EXP = mybir.ActivationFunctionType.Exp


def build_attention_nc(S=S_FULL):
    SB = S // P                 # s blocks
    SW = min(1024, S)           # stripe width (sq columns)
    NS = S // SW                # number of stripes
    nc = bass.Bass(trn_type="TRN2")

    x_d = nc.dram_tensor("x", [S, E], F32, kind="ExternalInput").ap()
    wq_d = nc.dram_tensor("Wq", [H, E, D], F32, kind="ExternalInput").ap()
    wk_d = nc.dram_tensor("Wk", [H, E, D], F32, kind="ExternalInput").ap()
    wv_d = nc.dram_tensor("Wv", [H, E, D], F32, kind="ExternalInput").ap()
    wp_d = nc.dram_tensor("Wp", [E, E], F32, kind="ExternalInput").ap()
    bp_d = nc.dram_tensor("bp", [E], F32, kind="ExternalInput").ap()
    out_d = nc.dram_tensor("out", [S, E], F32, kind="ExternalOutput").ap()

    with TileContext(nc) as tc, ExitStack() as top:
        const = top.enter_context(tc.tile_pool(name="const", bufs=1))
        warm = const.tile([1, 2], F32, tag="warm")
        nc.vector.memset(warm, 0.0)
        # pre-warm the exp table set (~2.7us) off the critical path
        nc.scalar.activation(out=warm[0:1, 1:2], in_=warm[0:1, 0:1], func=EXP)
        ident = const.tile([P, P], F32, tag="ident")
        make_identity(nc, ident)
        # mask01[sk, sq] = 0 where sq < sk else 1  (kills strictly-lower)
        mask01 = const.tile([P, P], BF16, tag="mask01")
        nc.gpsimd.memset(mask01, 1.0)
        nc.gpsimd.affine_select(
            out=mask01, in_=mask01, compare_op=mybir.AluOpType.is_ge,
            fill=0.0, base=0, pattern=[[1, P]], channel_multiplier=-1,
        )
        # row 64 used as the broadcast lhsT (must match pst's l-row base
        # partition for the matmul base-partition assert)
        ones = const.tile([D + 1, D], F32R, tag="ones")
        nc.vector.memset(ones.bitcast(F32), 1.0)
        bpb = const.tile([P, E], F32, tag="bpb")
        wp_sb = const.tile([P, G, E], BF16, tag="wp")

        out_pool = top.enter_context(tc.tile_pool(name="outsb", bufs=2))
        qkv = top.enter_context(tc.tile_pool(name="qkv", bufs=1))
        qT = [qkv.tile([P, S], BF16, tag=f"qT{g}", name=f"qT{g}") for g in range(G)]
        kT = [qkv.tile([P, S], BF16, tag=f"kT{g}", name=f"kT{g}") for g in range(G)]
        vt = qkv.tile([P, H, SB, 65], BF16, tag="vt")
        attp = top.enter_context(tc.tile_pool(name="attsb", bufs=3))

        # single PSUM pool: tag "pa" = working (2 banks x2), tag "po" =
        # attention output accumulators (2 banks x2)
        ppool = top.enter_context(tc.tile_pool(name="ppool", bufs=2, space="PSUM"))

        attout = top.enter_context(tc.tile_pool(name="attout", bufs=1))
        attoutT = [attout.tile([P, G, SW], BF16, tag=f"attoutT{t}",
                               name=f"attoutT{t}") for t in range(NS)]
        rpool = top.enter_context(tc.tile_pool(name="rp", bufs=2))

        # ---------------- attention (per stripe x head) + interleaved proj
        def emit_av(po, h, pend, lo, hi):
            sb, j = pend
            jlo = max(lo, j * P)
            for b in range(lo, hi, 512):
                clo, chi = max(jlo, b), b + 512
                if clo >= chi:
                    continue
                nc.tensor.matmul(po[:, clo - lo:chi - lo],
                                 lhsT=vt[:, h, j, :],
                                 rhs=sb[:, clo - lo:chi - lo],
                                 start=(j == 0), stop=(j == chi // P - 1))

        def emit_proj(si):
            tt, col = si * P // SW, (si * P) % SW
            pp = ppool.tile([P, E], F32, tag="pa", name="pp")
            for g in range(G):
                nc.tensor.matmul(pp, lhsT=attoutT[tt][:, g, col:col + P],
                                 rhs=wp_sb[:, g, :], start=(g == 0),
                                 stop=(g == G - 1))
            ob = out_pool.tile([P, E], F32, tag="ob", name="ob")
            nc.vector.tensor_add(out=ob, in0=pp, in1=bpb)
            nc.sync.dma_start(out=out_d[si * P:(si + 1) * P, :], in_=ob)

        def emit_att_head(t, h):
            lo, hi = t * SW, (t + 1) * SW
            jmax = hi // P
            g, hh = h // 2, (h % 2) * D
            po = ppool.tile([65, SW], F32, tag="po", name="po")
            pending = []
            for j in range(jmax):
                jlo = max(lo, j * P)
                ps = ppool.tile([P, SW], F32, tag="pa", name="ps")
                for b in range(lo, hi, 512):
                    clo, chi = max(jlo, b), b + 512
                    if clo >= chi:
                        continue
                    nc.tensor.matmul(ps[:, clo - lo:chi - lo],
                                     lhsT=kT[g][hh:hh + D, j * P:(j + 1) * P],
                                     rhs=qT[g][hh:hh + D, clo:chi],
                                     start=True, stop=True)
                if len(pending) >= 2:
                    emit_av(po, h, pending.pop(0), lo, hi)
                sb = attp.tile([P, SW], BF16, tag="attsb", name="sb")
                nc.scalar.activation(out=sb[:, jlo - lo:], in_=ps[:, jlo - lo:],
                                     func=EXP, scale=float(1.0 / np.sqrt(D)))
                if j * P >= lo:
                    # zero the strictly-lower triangle of the diagonal block
                    dcol = j * P - lo
                    nc.vector.tensor_mul(out=sb[:, dcol:dcol + P],
                                         in0=sb[:, dcol:dcol + P], in1=mask01)
                pending.append((sb, j))
            for pend in pending:
                emit_av(po, h, pend, lo, hi)
            # stage po into SBUF as f32r (frees the PSUM accumulator and
            # rounds l for the f32r broadcast matmul), then normalize:
            # broadcast l across 64 partitions on the PE, reciprocal on the
            # broadcast (per-partition parallel - same DVE cost as [1,SW]),
            # attoutT = pst * (1/l).
            pst = rpool.tile([D + 1, SW], F32R, tag="postage", name="pst")
            nc.vector.tensor_copy(out=pst, in_=po[0:D + 1, :])
            pbc = ppool.tile([D, SW], F32, tag="pa", name="pbc")
            for c in range(0, SW, 512):
                nc.tensor.matmul(pbc[:, c:c + 512], lhsT=ones[D:D + 1, :],
                                 rhs=pst[D:D + 1, c:c + 512],
                                 start=True, stop=True)
            rbc = rpool.tile([D, SW], F32, tag="rbc", name="rbc")
            nc.vector.reciprocal(out=rbc, in_=pbc)
            nc.vector.tensor_mul(out=attoutT[t][hh:hh + D, g, :],
                                 in0=pst[0:D, :].bitcast(F32), in1=rbc)

        with ExitStack() as ph2:
            xin = ph2.enter_context(tc.tile_pool(name="xin", bufs=2))
            xt_pool = ph2.enter_context(tc.tile_pool(name="xt", bufs=1))
            wpool = ph2.enter_context(tc.tile_pool(name="wqkv", bufs=1))

            xT = [xt_pool.tile([P, S], BF16, tag=f"xT{e}", name=f"xT{e}")
                  for e in range(EB)]

            # ---------------- batched input DMAs, x interleaved with weights
            wq_r = wq_d.rearrange("h e d -> e h d")
            wk_r = wk_d.rearrange("h e d -> e h d")
            wv_r = wv_d.rearrange("h e d -> e h d")
            wq_st = wpool.tile([P, EB, H, D], F32, tag="wqs", name="wqs")
            wk_st = wpool.tile([P, EB, H, D], F32, tag="wks", name="wks")
            wv_st = wpool.tile([P, EB, H, D], F32, tag="wvs", name="wvs")
            wp_st = wpool.tile([P, G, E], F32, tag="wps", name="wps")
            xs4 = []
            for k4 in range(SB // 4):
                xs = xin.tile([P, 4, E], F32, tag="xs", name=f"xs{k4}")
                nc.sync.dma_start(
                    out=xs,
                    in_=x_d[k4 * 512:(k4 + 1) * 512, :].rearrange(
                        "(i p) e -> p i e", p=P))
                xs4.append(xs)
                if k4 < 3:
                    w_r, w_st = ((wq_r, wq_st), (wk_r, wk_st), (wv_r, wv_st))[k4]
                    for ej in range(EB):
                        nc.sync.dma_start(out=w_st[:, ej],
                                          in_=w_r[ej * P:(ej + 1) * P, :, :])
            for g in range(G):
                nc.sync.dma_start(out=wp_st[:, g, :],
                                  in_=wp_d[g * P:(g + 1) * P, :])
            nc.sync.dma_start(
                out=bpb,
                in_=bass.AP(tensor=bp_d.tensor, offset=bp_d.offset,
                            ap=[[0, P]] + list(bp_d.ap)))

            # ---------------- x PE-transpose -> xT bf16 (copies split
            # between ScalarE and DVE)
            for k4 in range(SB // 4):
                for ej in range(EB):
                    pt = ppool.tile([P, 512], F32, tag="pa", name="pt")
                    for i in range(4):
                        nc.tensor.transpose(pt[:, i * P:(i + 1) * P],
                                            xs4[k4][:, i, ej * P:(ej + 1) * P],
                                            ident)
                    dst = xT[ej][:, k4 * 512:(k4 + 1) * 512]
                    if ej % 2 == 0:
                        nc.scalar.copy(out=dst, in_=pt)
                    else:
                        nc.vector.tensor_copy(out=dst, in_=pt)

            # ---------------- weight casts fp32 -> bf16
            wq_sb = wpool.tile([P, EB, H, D], BF16, tag="wq")
            wk_sb = wpool.tile([P, EB, H, D], BF16, tag="wk")
            wv_sb = wpool.tile([P, EB, H, D], BF16, tag="wv")
            for w_st, w_sb in ((wq_st, wq_sb), (wk_st, wk_sb), (wv_st, wv_sb)):
                nc.vector.tensor_copy(out=w_sb, in_=w_st)
            nc.vector.tensor_copy(out=wp_sb, in_=wp_st)

            # ---------------- v (all heads per s-block)
            nc.vector.memset(vt[:, :, :, 64:65], 1.0)
            for si in range(SB):
                pv = ppool.tile([P, E], F32, tag="pa", name="pv")
                for ej in range(EB):
                    nc.tensor.matmul(pv, lhsT=xT[ej][:, si * P:(si + 1) * P],
                                     rhs=wv_sb[:, ej], start=(ej == 0),
                                     stop=(ej == EB - 1))
                nc.vector.tensor_copy(out=vt[:, :, si, 0:64],
                                      in_=pv.rearrange("p (h d) -> p h d", h=H))

            # ---------------- qT / kT per group, interleaved with stripe-0
            # attention so exp starts early and the PE stays dense
            def emit_qkt(g):
                for w_sb, dst in ((wk_sb, kT[g]), (wq_sb, qT[g])):
                    for q0 in range(0, S, 1024):
                        pq = ppool.tile([P, 1024], F32, tag="pa", name="pq")
                        for ej in range(EB):
                            for c in range(q0, q0 + 1024, 512):
                                nc.tensor.matmul(
                                    pq[:, c - q0:c - q0 + 512],
                                    lhsT=w_sb[:, ej, 2 * g:2 * g + 2, :],
                                    rhs=xT[ej][:, c:c + 512],
                                    start=(ej == 0), stop=(ej == EB - 1))
                        nc.vector.tensor_copy(out=dst[:, q0:q0 + 1024], in_=pq)

            for g in range(G):
                emit_qkt(g)
                emit_att_head(0, 2 * g)
                emit_att_head(0, 2 * g + 1)

        for h in range(H):
            emit_att_head(1, h)
            emit_proj(h)
        for si in range(SW // P, S // P):
            emit_proj(si)

    _hoist_matmul_waits(nc)
    return nc


def _hoist_matmul_waits(nc):
    """Several TRN2 ISA structs (notably the fp32 self-loading Matmult's LW
    struct) accept only one sync-wait slot; walrus dies with "Too many sync
    wait commands" otherwise. Hoist every wait of a multi-wait instruction
    onto same-engine NoOps inserted right before it (same engine queue =>
    identical ordering semantics)."""
    nid = [0]
    for fn in nc.m.functions:
        for blk in fn.blocks:
            insts = blk.instructions
            out = []
            for inst in insts:
                si = inst.sync_info
                if (inst.engine != mybir.EngineType.Unassigned and si is not None
                        and len(si.on_wait) >= 2 and inst.opcode != "NoOp"):
                    for w in si.on_wait:
                        nid[0] += 1
                        nop = mybir.InstNoOp(name=f"I-mmwait-{nid[0]}",
                                             ins=[], outs=[])
                        nop.engine = inst.engine
                        nop.sync_info = mybir.SyncInfo(on_wait=[w], on_update=[])
                        nc.inst_map[nop.name] = nop
                        out.append(nop)
                    inst.sync_info = mybir.SyncInfo(on_wait=[],
                                                    on_update=list(si.on_update))
                out.append(inst)
            if len(out) != len(insts):
                insts[:] = out


_nc_cache = {}


def _get_nc(S=S_FULL):
    if S not in _nc_cache:
        _nc_cache[S] = build_attention_nc(S)
    return _nc_cache[S]


def kernel(x, Wq, Wk, Wv, Wp, bp, _trace=False):
    nc = _get_nc(x.shape[1])
    n = x.shape[0]
    wq = np.ascontiguousarray(Wq, np.float32)
    wk = np.ascontiguousarray(Wk, np.float32)
    wv = np.ascontiguousarray(Wv, np.float32)
    wp = np.ascontiguousarray(Wp, np.float32)
    bpc = np.ascontiguousarray(bp, np.float32)
    in_maps = [
        {"x": np.ascontiguousarray(x[b], np.float32),
         "Wq": wq, "Wk": wk, "Wv": wv, "Wp": wp, "bp": bpc}
        for b in range(n)
    ]
    res = bass_utils.run_bass_kernel_spmd(
        nc, in_maps, core_ids=list(range(n)), trace=_trace)
    out = np.stack([r["out"] for r in res.results], axis=0)
    if _trace:
        return out, res
    return out


# revision 24
# speedup vs baseline: 1.2990x; 1.2990x over previous
"""Multi-head causal attention (B=8, S=2048, E=512, H=8, D=64) on 8 trn2 cores.

Strategy: pure data parallelism over the batch dimension (B == n_cores == 8).
Each NeuronCore computes the full attention for one batch element; no
collectives are needed.

v2 (vs baseline): attention matmul operands in bf16 (1 cyc/row on PE at any
width; fp32r pays 4x under 256 cols), x loaded in 4 batched DMAs instead of
16 (DMA-issue on the sync queue costs ~626ns each), qkT emission interleaved
with stripe-0 attention so ScalarE's exp stream starts early and the PE never
idles long enough for the HAM clock gate to re-throttle to 1.2 GHz, diagonal
causal mask applied as a DVE multiply with a precomputed 0/1 bf16 mask
(instead of a PE mask-matmul), softmax normalization via
reciprocal_approx_fast + PE ones-broadcast (instead of full-precision DVE
reciprocal at ~6.5us and a DRAM round-trip), PSUM->SBUF staging copy and
proj bias-add on GpSimd (idle otherwise), and the exp activation table
pre-warmed at kernel start.

Per core:
  1. Load x [S,E] (4 batched DMAs), transpose on PE -> xT bf16 [E,S].
  2. QKV: qT/kT per 2-head group g from wq/wk (bf16), v stored interleaved
     as vt[p=sk, h, j, 0:64] bf16 with vt[..., 64] = 1.0 so the AV matmul's
     stationary [128, 65] also produces the softmax denominator (row 64).
  3. Per sq-stripe (1024) and head: scoresT[sk,sq] = kT_j^T @ qT (K=64,
     bf16), exp via ScalarE (scale=1/8, no max subtraction needed:
     |scores/8| <~ 2) -> sb bf16, diag block masked by multiplying with a
     strictly-lower-zero mask on DVE, outT_aug[65,sq] += vt_j^T @ attT_j.
     Software-pipelined two deep so the PE does not stall on exp.
  4. Normalize: pst = po staged to SBUF (GpSimd, frees the PSUM bank),
     r = 1/l via reciprocal_approx_fast (DVE), broadcast r across 64
     partitions with a PE ones-matmul, attoutT = pst * r_bcast (DVE, bf16).
  5. Output projection out[s,e] = attoutT^T @ Wp (bf16) + bp (GpSimd add)
     -> HBM, interleaved with the next stripe's attention.

A single PSUM pool with two tags (4+4 banks) is used for the whole kernel.
Post-scheduling, excess semaphore waits are hoisted onto same-engine NoOps
(several ISA structs accept only one wait slot; walrus rejects multi-wait
instructions).
"""

import numpy as np
from contextlib import ExitStack

import concourse.bass as bass
import concourse.mybir as mybir
from concourse.tile import TileContext
from concourse.masks import make_identity
from concourse import bass_utils

F32 = mybir.dt.float32
F32R = mybir.dt.float32r
BF16 = mybir.dt.bfloat16
B, S_FULL, E, H, D = 8, 2048, 512, 8, 64
P = 128
G = H // 2      # 2-head groups
EB = E // P     # e blocks
EXP = mybir.ActivationFunctionType.Exp


def build_attention_nc(S=S_FULL):
    SB = S // P                 # s blocks
    SW = min(1024, S)           # stripe width (sq columns)
    NS = S // SW                # number of stripes
    nc = bass.Bass(trn_type="TRN2")

    x_d = nc.dram_tensor("x", [S, E], F32, kind="ExternalInput").ap()
    wq_d = nc.dram_tensor("Wq", [H, E, D], F32, kind="ExternalInput").ap()
    wk_d = nc.dram_tensor("Wk", [H, E, D], F32, kind="ExternalInput").ap()
    wv_d = nc.dram_tensor("Wv", [H, E, D], F32, kind="ExternalInput").ap()
    wp_d = nc.dram_tensor("Wp", [E, E], F32, kind="ExternalInput").ap()
    bp_d = nc.dram_tensor("bp", [E], F32, kind="ExternalInput").ap()
    out_d = nc.dram_tensor("out", [S, E], F32, kind="ExternalOutput").ap()
    # scratch for the batched softmax-denominator reciprocal: l rows go out
    # in row layout, come back partition-spread [128, H*SW/128], and return
    # reciprocal'd in row layout for the PE broadcast matmul.
    lscr = nc.dram_tensor("lscr", [NS * H * SW], BF16, kind="Internal").ap()
    rscr = nc.dram_tensor("rscr", [NS * H * SW], BF16, kind="Internal").ap()

    with TileContext(nc) as tc, ExitStack() as top:
        const = top.enter_context(tc.tile_pool(name="const", bufs=1))
        warm = const.tile([1, 2], F32, tag="warm")
        nc.vector.memset(warm, 0.0)
        # pre-warm the exp table set (~2.7us) off the critical path
        nc.scalar.activation(out=warm[0:1, 1:2], in_=warm[0:1, 0:1], func=EXP)
        ident = const.tile([P, P], F32, tag="ident")
        make_identity(nc, ident)
        ones = const.tile([1, D], BF16, tag="ones")
        nc.vector.memset(ones, 1.0)
        bpb = const.tile([P, E], F32, tag="bpb")
        wp_sb = const.tile([P, G, E], BF16, tag="wp")

        out_pool = top.enter_context(tc.tile_pool(name="outsb", bufs=2))
        qkv = top.enter_context(tc.tile_pool(name="qkv", bufs=1))
        qT = [qkv.tile([P, S], BF16, tag=f"qT{g}", name=f"qT{g}") for g in range(G)]
        kT = [qkv.tile([P, S], BF16, tag=f"kT{g}", name=f"kT{g}") for g in range(G)]
        vt = qkv.tile([P, H, SB, 65], BF16, tag="vt")
        attp = top.enter_context(tc.tile_pool(name="attsb", bufs=3))

        # single PSUM pool: tag "pa" = working (2 banks x2), tag "po" =
        # attention output accumulators (2 banks x2)
        ppool = top.enter_context(tc.tile_pool(name="ppool", bufs=2, space="PSUM"))

        attout = top.enter_context(tc.tile_pool(name="attout", bufs=1))
        attoutT = [attout.tile([P, G, SW], BF16, tag=f"attoutT{t}",
                               name=f"attoutT{t}") for t in range(NS)]
        rpool = top.enter_context(tc.tile_pool(name="rp", bufs=2))
        # r rows are DMA'd back into the same strip the l rows left from
        # (WAR tracked by the tile layer; saves 32KB/partition of SBUF)
        lrow = [attout.tile([1, H * SW], BF16, tag=f"lrow{t}", name=f"lrow{t}")
                for t in range(NS)]
        rrow = lrow

        # ---------------- attention (per stripe x head) + interleaved proj
        def emit_av(po, h, pend, lo, hi):
            sb, j = pend
            jlo = max(lo, j * P)
            for b in range(lo, hi, 512):
                clo, chi = max(jlo, b), b + 512
                if clo >= chi:
                    continue
                nc.tensor.matmul(po[:, clo - lo:chi - lo],
                                 lhsT=vt[:, h, j, :],
                                 rhs=sb[:, clo - lo:chi - lo],
                                 start=(j == 0), stop=(j == chi // P - 1))

        def emit_proj(si):
            tt, col = si * P // SW, (si * P) % SW
            pp = ppool.tile([P, E], F32, tag="pa", name="pp")
            for g in range(G):
                nc.tensor.matmul(pp, lhsT=attoutT[tt][:, g, col:col + P],
                                 rhs=wp_sb[:, g, :], start=(g == 0),
                                 stop=(g == G - 1))
            ob = out_pool.tile([P, E], F32, tag="ob", name="ob")
            nc.vector.tensor_add(out=ob, in0=pp, in1=bpb)
            nc.sync.dma_start(out=out_d[si * P:(si + 1) * P, :], in_=ob)

        def emit_att_head(t, h):
            lo, hi = t * SW, (t + 1) * SW
            jmax = hi // P
            g, hh = h // 2, (h % 2) * D
            po = ppool.tile([65, SW], F32, tag="po", name="po")
            pending = []
            for j in range(jmax):
                jlo = max(lo, j * P)
                ps = ppool.tile([P, SW], F32, tag="pa", name="ps")
                for b in range(lo, hi, 512):
                    clo, chi = max(jlo, b), b + 512
                    if clo >= chi:
                        continue
                    nc.tensor.matmul(ps[:, clo - lo:chi - lo],
                                     lhsT=kT[g][hh:hh + D, j * P:(j + 1) * P],
                                     rhs=qT[g][hh:hh + D, clo:chi],
                                     start=True, stop=True)
                if len(pending) >= 2:
                    emit_av(po, h, pending.pop(0), lo, hi)
                sb = attp.tile([P, SW], BF16, tag="attsb", name="sb")
                nc.scalar.activation(out=sb[:, jlo - lo:], in_=ps[:, jlo - lo:],
                                     func=EXP, scale=float(1.0 / np.sqrt(D)))
                if j * P >= lo:
                    # zero the strictly-lower triangle of the diagonal block
                    # (on GpSimd: keeps it out of the DVE FIFO, which the
                    # AV matmuls depend on via the norm-chain ops)
                    dcol = j * P - lo
                    nc.gpsimd.affine_select(
                        out=sb[:, dcol:dcol + P], in_=sb[:, dcol:dcol + P],
                        compare_op=mybir.AluOpType.is_ge, fill=0.0,
                        base=0, pattern=[[1, P]], channel_multiplier=-1)
                pending.append((sb, j))
            for pend in pending:
                emit_av(po, h, pend, lo, hi)
            # store the UNNORMALIZED attention output (scaled in place once
            # the batched reciprocal row returns) and stage the denominator
            # row l into this stripe's row-layout staging strip (f32r copy
            # rounds it for the f32r broadcast matmul).
            nc.vector.tensor_copy(out=attoutT[t][hh:hh + D, g, :],
                                  in_=po[0:D, :])
            nc.vector.tensor_copy(out=lrow[t][0:1, h * SW:(h + 1) * SW],
                                  in_=po[D:D + 1, :])

        def emit_stripe_recip(t):
            # all H denominator rows of stripe t -> DRAM -> partition-spread
            # [128, H*SW/128] -> one cheap DVE reciprocal -> DRAM -> row
            # layout for the per-head PE broadcast.
            n = H * SW
            off = t * n
            nc.sync.dma_start(
                out=bass.AP(tensor=lscr.tensor, offset=lscr.offset + off,
                            ap=[[0, 1], [1, n]]),
                in_=lrow[t])
            lsp = rpool.tile([P, n // P], BF16, tag="lsp", name="lsp")
            nc.sync.dma_start(
                out=lsp,
                in_=bass.AP(tensor=lscr.tensor, offset=lscr.offset + off,
                            ap=[[n // P, P], [1, n // P]]))
            rsp = rpool.tile([P, n // P], BF16, tag="rsp", name="rsp")
            with nc.allow_low_precision("softmax denom reciprocal; rel-err "
                                        "budget 2e-2 >> bf16 eps"):
                nc.vector.reciprocal(out=rsp, in_=lsp)
            nc.sync.dma_start(
                out=bass.AP(tensor=rscr.tensor, offset=rscr.offset + off,
                            ap=[[n // P, P], [1, n // P]]),
                in_=rsp)
            nc.sync.dma_start(
                out=rrow[t],
                in_=bass.AP(tensor=rscr.tensor, offset=rscr.offset + off,
                            ap=[[0, 1], [1, n]]))

        def emit_norm(t, h):
            # attoutT[t] *= bcast(1/l) in place
            g, hh = h // 2, (h % 2) * D
            pbc = ppool.tile([D, SW], F32, tag="pa", name="pbc")
            for c in range(0, SW, 512):
                nc.tensor.matmul(
                    pbc[:, c:c + 512], lhsT=ones,
                    rhs=rrow[t][0:1, h * SW + c:h * SW + c + 512],
                    start=True, stop=True)
            sl = attoutT[t][hh:hh + D, g, :]
            nc.vector.tensor_mul(out=sl, in0=sl, in1=pbc)

        with ExitStack() as ph2:
            xin = ph2.enter_context(tc.tile_pool(name="xin", bufs=2))
            xt_pool = ph2.enter_context(tc.tile_pool(name="xt", bufs=1))
            wpool = ph2.enter_context(tc.tile_pool(name="wqkv", bufs=1))

            xT = [xt_pool.tile([P, S], BF16, tag=f"xT{e}", name=f"xT{e}")
                  for e in range(EB)]

            # ---------------- batched input DMAs, x interleaved with weights
            wq_r = wq_d.rearrange("h e d -> e h d")
            wk_r = wk_d.rearrange("h e d -> e h d")
            wv_r = wv_d.rearrange("h e d -> e h d")
            wq_st = wpool.tile([P, EB, H, D], F32, tag="wqs", name="wqs")
            wk_st = wpool.tile([P, EB, H, D], F32, tag="wks", name="wks")
            wv_st = wpool.tile([P, EB, H, D], F32, tag="wvs", name="wvs")
            wp_st = wpool.tile([P, G, E], F32, tag="wps", name="wps")
            xs4 = []
            for k4 in range(SB // 4):
                xs = xin.tile([P, 4, E], F32, tag="xs", name=f"xs{k4}")
                # alternate DMA issue queues: Sync and DVE program their DGEs
                # in parallel instead of serializing ~650ns issues on Sync
                dma_eng = nc.sync if k4 % 2 == 0 else nc.scalar
                dma_eng.dma_start(
                    out=xs,
                    in_=x_d[k4 * 512:(k4 + 1) * 512, :].rearrange(
                        "(i p) e -> p i e", p=P))
                xs4.append(xs)
                if k4 < 3:
                    w_r, w_st = ((wq_r, wq_st), (wk_r, wk_st), (wv_r, wv_st))[k4]
                    for ej in range(EB):
                        nc.sync.dma_start(out=w_st[:, ej],
                                          in_=w_r[ej * P:(ej + 1) * P, :, :])
            for g in range(G):
                nc.sync.dma_start(out=wp_st[:, g, :],
                                  in_=wp_d[g * P:(g + 1) * P, :])
            nc.sync.dma_start(
                out=bpb,
                in_=bass.AP(tensor=bp_d.tensor, offset=bp_d.offset,
                            ap=[[0, P]] + list(bp_d.ap)))

            # ---------------- x PE-transpose -> xT bf16 (copies split
            # between ScalarE and DVE)
            for k4 in range(SB // 4):
                for ej in range(EB):
                    pt = ppool.tile([P, 512], F32, tag="pa", name="pt")
                    for i in range(4):
                        nc.tensor.transpose(pt[:, i * P:(i + 1) * P],
                                            xs4[k4][:, i, ej * P:(ej + 1) * P],
                                            ident)
                    dst = xT[ej][:, k4 * 512:(k4 + 1) * 512]
                    if ej % 2 == 0:
                        nc.scalar.copy(out=dst, in_=pt)
                    else:
                        nc.vector.tensor_copy(out=dst, in_=pt)

            # ---------------- weight casts fp32 -> bf16
            wq_sb = wpool.tile([P, EB, H, D], BF16, tag="wq")
            wk_sb = wpool.tile([P, EB, H, D], BF16, tag="wk")
            wv_sb = wpool.tile([P, EB, H, D], BF16, tag="wv")
            for w_st, w_sb in ((wq_st, wq_sb), (wk_st, wk_sb), (wv_st, wv_sb)):
                nc.vector.tensor_copy(out=w_sb, in_=w_st)
            nc.vector.tensor_copy(out=wp_sb, in_=wp_st)

            # ---------------- v (all heads per s-block)
            nc.vector.memset(vt[:, :, :, 64:65], 1.0)
            for si in range(SB):
                pv = ppool.tile([P, E], F32, tag="pa", name="pv")
                for ej in range(EB):
                    nc.tensor.matmul(pv, lhsT=xT[ej][:, si * P:(si + 1) * P],
                                     rhs=wv_sb[:, ej], start=(ej == 0),
                                     stop=(ej == EB - 1))
                nc.vector.tensor_copy(out=vt[:, :, si, 0:64],
                                      in_=pv.rearrange("p (h d) -> p h d", h=H))

            # ---------------- qT / kT per group, interleaved with stripe-0
            # attention so exp starts early and the PE stays dense
            def emit_qkt(g):
                for w_sb, dst in ((wk_sb, kT[g]), (wq_sb, qT[g])):
                    for q0 in range(0, S, 1024):
                        pq = ppool.tile([P, 1024], F32, tag="pa", name="pq")
                        for ej in range(EB):
                            for c in range(q0, q0 + 1024, 512):
                                nc.tensor.matmul(
                                    pq[:, c - q0:c - q0 + 512],
                                    lhsT=w_sb[:, ej, 2 * g:2 * g + 2, :],
                                    rhs=xT[ej][:, c:c + 512],
                                    start=(ej == 0), stop=(ej == EB - 1))
                        nc.vector.tensor_copy(out=dst[:, q0:q0 + 1024], in_=pq)

            for g in range(G):
                emit_qkt(g)
                emit_att_head(0, 2 * g)
                emit_att_head(0, 2 * g + 1)

        emit_stripe_recip(0)
        for h in range(H):
            emit_att_head(1, h)
            emit_norm(0, h)
        emit_stripe_recip(1)
        for si in range(SW // P):
            emit_proj(si)
        for h in range(H):
            emit_norm(1, h)
        for si in range(SW // P, S // P):
            emit_proj(si)

    _hoist_matmul_waits(nc)
    return nc


def _hoist_matmul_waits(nc):
    """Several TRN2 ISA structs (notably the fp32 self-loading Matmult's LW
    struct) accept only one sync-wait slot; walrus dies with "Too many sync
    wait commands" otherwise. Hoist every wait of a multi-wait instruction
    onto same-engine NoOps inserted right before it (same engine queue =>
    identical ordering semantics)."""
    nid = [0]
    for fn in nc.m.functions:
        for blk in fn.blocks:
            insts = blk.instructions
            out = []
            for inst in insts:
                si = inst.sync_info
                if (inst.engine != mybir.EngineType.Unassigned and si is not None
                        and len(si.on_wait) >= 2 and inst.opcode != "NoOp"):
                    for w in si.on_wait:
                        nid[0] += 1
                        nop = mybir.InstNoOp(name=f"I-mmwait-{nid[0]}",
                                             ins=[], outs=[])
                        nop.engine = inst.engine
                        nop.sync_info = mybir.SyncInfo(on_wait=[w], on_update=[])
                        nc.inst_map[nop.name] = nop
                        out.append(nop)
                    inst.sync_info = mybir.SyncInfo(on_wait=[],
                                                    on_update=list(si.on_update))
                out.append(inst)
            if len(out) != len(insts):
                insts[:] = out


_nc_cache = {}


def _get_nc(S=S_FULL):
    if S not in _nc_cache:
        _nc_cache[S] = build_attention_nc(S)
    return _nc_cache[S]


def kernel(x, Wq, Wk, Wv, Wp, bp, _trace=False):
    nc = _get_nc(x.shape[1])
    n = x.shape[0]
    wq = np.ascontiguousarray(Wq, np.float32)
    wk = np.ascontiguousarray(Wk, np.float32)
    wv = np.ascontiguousarray(Wv, np.float32)
    wp = np.ascontiguousarray(Wp, np.float32)
    bpc = np.ascontiguousarray(bp, np.float32)
    in_maps = [
        {"x": np.ascontiguousarray(x[b], np.float32),
         "Wq": wq, "Wk": wk, "Wv": wv, "Wp": wp, "bp": bpc}
        for b in range(n)
    ]
    res = bass_utils.run_bass_kernel_spmd(
        nc, in_maps, core_ids=list(range(n)), trace=_trace)
    out = np.stack([r["out"] for r in res.results], axis=0)
    if _trace:
        return out, res
    return out


# revision 30
# speedup vs baseline: 1.3995x; 1.0774x over previous
"""Multi-head causal attention (B=8, S=2048, E=512, H=8, D=64) on 8 trn2 cores.

Strategy: pure data parallelism over the batch dimension (B == n_cores == 8).
Each NeuronCore computes the full attention for one batch element; no
collectives are needed.

v2 (vs baseline): attention matmul operands in bf16 (1 cyc/row on PE at any
width; fp32r pays 4x under 256 cols), x loaded in 4 batched DMAs instead of
16 (DMA-issue on the sync queue costs ~626ns each), qkT emission interleaved
with stripe-0 attention so ScalarE's exp stream starts early and the PE never
idles long enough for the HAM clock gate to re-throttle to 1.2 GHz, diagonal
causal mask applied as a DVE multiply with a precomputed 0/1 bf16 mask
(instead of a PE mask-matmul), softmax normalization via
reciprocal_approx_fast + PE ones-broadcast (instead of full-precision DVE
reciprocal at ~6.5us and a DRAM round-trip), PSUM->SBUF staging copy and
proj bias-add on GpSimd (idle otherwise), and the exp activation table
pre-warmed at kernel start.

Per core:
  1. Load x [S,E] (4 batched DMAs), transpose on PE -> xT bf16 [E,S].
  2. QKV: qT/kT per 2-head group g from wq/wk (bf16), v stored interleaved
     as vt[p=sk, h, j, 0:64] bf16 with vt[..., 64] = 1.0 so the AV matmul's
     stationary [128, 65] also produces the softmax denominator (row 64).
  3. Per sq-stripe (1024) and head: scoresT[sk,sq] = kT_j^T @ qT (K=64,
     bf16), exp via ScalarE (scale=1/8, no max subtraction needed:
     |scores/8| <~ 2) -> sb bf16, diag block masked by multiplying with a
     strictly-lower-zero mask on DVE, outT_aug[65,sq] += vt_j^T @ attT_j.
     Software-pipelined two deep so the PE does not stall on exp.
  4. Normalize: pst = po staged to SBUF (GpSimd, frees the PSUM bank),
     r = 1/l via reciprocal_approx_fast (DVE), broadcast r across 64
     partitions with a PE ones-matmul, attoutT = pst * r_bcast (DVE, bf16).
  5. Output projection out[s,e] = attoutT^T @ Wp (bf16) + bp (GpSimd add)
     -> HBM, interleaved with the next stripe's attention.

A single PSUM pool with two tags (4+4 banks) is used for the whole kernel.
Post-scheduling, excess semaphore waits are hoisted onto same-engine NoOps
(several ISA structs accept only one wait slot; walrus rejects multi-wait
instructions).
"""

import numpy as np
from contextlib import ExitStack

import concourse.bass as bass
import concourse.mybir as mybir
from concourse.tile import TileContext
from concourse.masks import make_identity
from concourse import bass_utils

F32 = mybir.dt.float32
F32R = mybir.dt.float32r
BF16 = mybir.dt.bfloat16
B, S_FULL, E, H, D = 8, 2048, 512, 8, 64
P = 128
G = H // 2      # 2-head groups
EB = E // P     # e blocks
EXP = mybir.ActivationFunctionType.Exp


def build_attention_nc(S=S_FULL):
    SB = S // P                 # s blocks
    SW = min(1024, S)           # stripe width (sq columns)
    NS = S // SW                # number of stripes
    nc = bass.Bass(trn_type="TRN2")

    x_d = nc.dram_tensor("x", [S, E], F32, kind="ExternalInput").ap()
    wq_d = nc.dram_tensor("Wq", [H, E, D], F32, kind="ExternalInput").ap()
    wk_d = nc.dram_tensor("Wk", [H, E, D], F32, kind="ExternalInput").ap()
    wv_d = nc.dram_tensor("Wv", [H, E, D], F32, kind="ExternalInput").ap()
    wp_d = nc.dram_tensor("Wp", [E, E], F32, kind="ExternalInput").ap()
    bp_d = nc.dram_tensor("bp", [E], F32, kind="ExternalInput").ap()
    out_d = nc.dram_tensor("out", [S, E], F32, kind="ExternalOutput").ap()
    # scratch for the batched softmax-denominator reciprocal: l rows go out
    # in row layout, come back partition-spread [128, H*SW/128], and return
    # reciprocal'd in row layout for the PE broadcast matmul.
    lscr = nc.dram_tensor("lscr", [NS * H * SW], BF16, kind="Internal").ap()
    rscr = nc.dram_tensor("rscr", [NS * H * SW], BF16, kind="Internal").ap()

    with TileContext(nc) as tc, ExitStack() as top:
        const = top.enter_context(tc.tile_pool(name="const", bufs=1))
        warm = const.tile([1, 2], F32, tag="warm")
        nc.vector.memset(warm, 0.0)
        # pre-warm the exp table set (~2.7us) off the critical path
        nc.scalar.activation(out=warm[0:1, 1:2], in_=warm[0:1, 0:1], func=EXP)
        ident = const.tile([P, P], F32, tag="ident")
        make_identity(nc, ident)
        ones = const.tile([1, D], BF16, tag="ones")
        nc.vector.memset(ones, 1.0)
        bpb = const.tile([P, E], F32, tag="bpb")
        wp_sb = const.tile([P, G, E], BF16, tag="wp")

        out_pool = top.enter_context(tc.tile_pool(name="outsb", bufs=2))
        qkv = top.enter_context(tc.tile_pool(name="qkv", bufs=1))
        qT = [qkv.tile([P, S], BF16, tag=f"qT{g}", name=f"qT{g}") for g in range(G)]
        kT = [qkv.tile([P, S], BF16, tag=f"kT{g}", name=f"kT{g}") for g in range(G)]
        vt = qkv.tile([P, H, SB, 65], BF16, tag="vt")
        attp = top.enter_context(tc.tile_pool(name="attsb", bufs=4))

        # single PSUM pool: tag "pa" = working (2 banks x3, lets scores run
        # 3 j-blocks ahead of exp), tag "po" = attention output accumulator
        # (2 banks x1; freed by the post-AV copies before the next head's
        # first AV needs it)
        ppool = top.enter_context(tc.tile_pool(name="ppool", bufs=3, space="PSUM"))

        attout = top.enter_context(tc.tile_pool(name="attout", bufs=1))
        attoutT = [attout.tile([P, G, SW], BF16, tag=f"attoutT{t}",
                               name=f"attoutT{t}") for t in range(NS)]
        rpool = top.enter_context(tc.tile_pool(name="rp", bufs=2))
        # r rows are DMA'd back into the same strip the l rows left from
        # (WAR tracked by the tile layer; saves 32KB/partition of SBUF)
        lrow = [attout.tile([1, H * SW], BF16, tag=f"lrow{t}", name=f"lrow{t}")
                for t in range(NS)]
        rrow = lrow

        # ---------------- attention (per stripe x head) + interleaved proj
        def emit_av(po, h, pend, lo, hi):
            sb, j = pend
            jlo = max(lo, j * P)
            for b in range(lo, hi, 512):
                clo, chi = max(jlo, b), b + 512
                if clo >= chi:
                    continue
                nc.tensor.matmul(po[:, clo - lo:chi - lo],
                                 lhsT=vt[:, h, j, :],
                                 rhs=sb[:, clo - lo:chi - lo],
                                 start=(j == 0), stop=(j == chi // P - 1))

        def emit_proj(si):
            tt, col = si * P // SW, (si * P) % SW
            pp = ppool.tile([P, E], F32, tag="pa", name="pp")
            for g in range(G):
                nc.tensor.matmul(pp, lhsT=attoutT[tt][:, g, col:col + P],
                                 rhs=wp_sb[:, g, :], start=(g == 0),
                                 stop=(g == G - 1))
            ob = out_pool.tile([P, E], F32, tag="ob", name="ob")
            nc.vector.tensor_add(out=ob, in0=pp, in1=bpb)
            nc.sync.dma_start(out=out_d[si * P:(si + 1) * P, :], in_=ob)

        def emit_att_head(t, h):
            lo, hi = t * SW, (t + 1) * SW
            jmax = hi // P
            g, hh = h // 2, (h % 2) * D
            po = ppool.tile([65, SW], F32, tag="po", name="po", bufs=1)
            pending = []
            for j in range(jmax):
                jlo = max(lo, j * P)
                ps = ppool.tile([P, SW], F32, tag="pa", name="ps")
                for b in range(lo, hi, 512):
                    clo, chi = max(jlo, b), b + 512
                    if clo >= chi:
                        continue
                    nc.tensor.matmul(ps[:, clo - lo:chi - lo],
                                     lhsT=kT[g][hh:hh + D, j * P:(j + 1) * P],
                                     rhs=qT[g][hh:hh + D, clo:chi],
                                     start=True, stop=True)
                if len(pending) >= 3:
                    emit_av(po, h, pending.pop(0), lo, hi)
                sb = attp.tile([P, SW], BF16, tag="attsb", name="sb")
                nc.scalar.activation(out=sb[:, jlo - lo:], in_=ps[:, jlo - lo:],
                                     func=EXP, scale=float(1.0 / np.sqrt(D)))
                if j * P >= lo:
                    # zero the strictly-lower triangle of the diagonal block
                    # (on GpSimd: keeps it out of the DVE FIFO, which the
                    # AV matmuls depend on via the norm-chain ops)
                    dcol = j * P - lo
                    nc.gpsimd.affine_select(
                        out=sb[:, dcol:dcol + P], in_=sb[:, dcol:dcol + P],
                        compare_op=mybir.AluOpType.is_ge, fill=0.0,
                        base=0, pattern=[[1, P]], channel_multiplier=-1)
                pending.append((sb, j))
            for pend in pending:
                emit_av(po, h, pend, lo, hi)
            # store the UNNORMALIZED attention output (scaled in place once
            # the batched reciprocal row returns) and stage the denominator
            # row l into this stripe's row-layout staging strip (f32r copy
            # rounds it for the f32r broadcast matmul).
            nc.vector.tensor_copy(out=attoutT[t][hh:hh + D, g, :],
                                  in_=po[0:D, :])
            nc.vector.tensor_copy(out=lrow[t][0:1, h * SW:(h + 1) * SW],
                                  in_=po[D:D + 1, :])

        def emit_stripe_recip(t):
            # all H denominator rows of stripe t -> DRAM -> partition-spread
            # [128, H*SW/128] -> one cheap DVE reciprocal -> DRAM -> row
            # layout for the per-head PE broadcast.
            n = H * SW
            off = t * n
            nc.sync.dma_start(
                out=bass.AP(tensor=lscr.tensor, offset=lscr.offset + off,
                            ap=[[0, 1], [1, n]]),
                in_=lrow[t])
            lsp = rpool.tile([P, n // P], BF16, tag="lsp", name="lsp")
            nc.sync.dma_start(
                out=lsp,
                in_=bass.AP(tensor=lscr.tensor, offset=lscr.offset + off,
                            ap=[[n // P, P], [1, n // P]]))
            rsp = rpool.tile([P, n // P], BF16, tag="rsp", name="rsp")
            with nc.allow_low_precision("softmax denom reciprocal; rel-err "
                                        "budget 2e-2 >> bf16 eps"):
                nc.vector.reciprocal(out=rsp, in_=lsp)
            nc.sync.dma_start(
                out=bass.AP(tensor=rscr.tensor, offset=rscr.offset + off,
                            ap=[[n // P, P], [1, n // P]]),
                in_=rsp)
            nc.sync.dma_start(
                out=rrow[t],
                in_=bass.AP(tensor=rscr.tensor, offset=rscr.offset + off,
                            ap=[[0, 1], [1, n]]))

        def emit_norm(t, h):
            # attoutT[t] *= bcast(1/l) in place
            g, hh = h // 2, (h % 2) * D
            pbc = ppool.tile([D, SW], F32, tag="pa", name="pbc")
            for c in range(0, SW, 512):
                nc.tensor.matmul(
                    pbc[:, c:c + 512], lhsT=ones,
                    rhs=rrow[t][0:1, h * SW + c:h * SW + c + 512],
                    start=True, stop=True)
            sl = attoutT[t][hh:hh + D, g, :]
            nc.vector.tensor_mul(out=sl, in0=sl, in1=pbc)

        with ExitStack() as ph2:
            xin = ph2.enter_context(tc.tile_pool(name="xin", bufs=2))
            xt_pool = ph2.enter_context(tc.tile_pool(name="xt", bufs=1))
            wpool = ph2.enter_context(tc.tile_pool(name="wqkv", bufs=1))

            xT = [xt_pool.tile([P, S], BF16, tag=f"xT{e}", name=f"xT{e}")
                  for e in range(EB)]

            # ---------------- batched input DMAs, x interleaved with weights
            wq_r = wq_d.rearrange("h e d -> e h d")
            wk_r = wk_d.rearrange("h e d -> e h d")
            wv_r = wv_d.rearrange("h e d -> e h d")
            wq_st = wpool.tile([P, EB, H, D], F32, tag="wqs", name="wqs")
            wk_st = wpool.tile([P, EB, H, D], F32, tag="wks", name="wks")
            wv_st = wpool.tile([P, EB, H, D], F32, tag="wvs", name="wvs")
            wp_st = wpool.tile([P, G, E], F32, tag="wps", name="wps")
            # x in 8 half-MB chunks alternating the Sync/Scalar DMA queues
            # (two DGEs program in parallel; first chunk lands fast so the
            # PE transposes start early). wv is loaded first among the
            # weights (the v matmuls are the first weight consumers).
            xs4 = [xin.tile([P, 4, E], F32, tag="xs", name=f"xs{k4}")
                   for k4 in range(SB // 4)]
            w_dmas = []
            for w_r, w_st in ((wv_r, wv_st), (wq_r, wq_st), (wk_r, wk_st)):
                for ej in range(EB):
                    w_dmas.append((w_st[:, ej], w_r[ej * P:(ej + 1) * P, :, :]))
            for g in range(G):
                w_dmas.append((wp_st[:, g, :], wp_d[g * P:(g + 1) * P, :]))
            wi = 0
            for k8 in range(SB // 2):
                k4, i2 = k8 // 2, (k8 % 2) * 2
                eng = nc.sync if k8 % 2 == 0 else nc.scalar
                eng.dma_start(
                    out=xs4[k4][:, i2:i2 + 2, :],
                    in_=x_d[k8 * 256:(k8 + 1) * 256, :].rearrange(
                        "(i p) e -> p i e", p=P))
                if k8 >= 1 and wi < len(w_dmas):
                    # two weight DMAs per x chunk, on the other queue
                    oeng = nc.scalar if k8 % 2 == 0 else nc.sync
                    for _ in range(2):
                        if wi < len(w_dmas):
                            dst, src = w_dmas[wi]
                            oeng.dma_start(out=dst, in_=src)
                            wi += 1
            while wi < len(w_dmas):
                dst, src = w_dmas[wi]
                nc.scalar.dma_start(out=dst, in_=src)
                wi += 1
            nc.sync.dma_start(
                out=bpb,
                in_=bass.AP(tensor=bp_d.tensor, offset=bp_d.offset,
                            ap=[[0, P]] + list(bp_d.ap)))

            # ---------------- weight casts fp32 -> bf16 (wv first: it lands
            # first and the v matmuls are its only consumer; wq/wk/wp casts
            # go after the xT copies so they don't block the DVE FIFO)
            wq_sb = wpool.tile([P, EB, H, D], BF16, tag="wq")
            wk_sb = wpool.tile([P, EB, H, D], BF16, tag="wk")
            wv_sb = wpool.tile([P, EB, H, D], BF16, tag="wv")
            nc.vector.tensor_copy(out=wv_sb, in_=wv_st)

            # ---------------- x PE-transpose -> xT bf16 (copies split
            # between ScalarE and DVE)
            for k4 in range(SB // 4):
                for ej in range(EB):
                    pt = ppool.tile([P, 512], F32, tag="pa", name="pt")
                    for i in range(4):
                        nc.tensor.transpose(pt[:, i * P:(i + 1) * P],
                                            xs4[k4][:, i, ej * P:(ej + 1) * P],
                                            ident)
                    dst = xT[ej][:, k4 * 512:(k4 + 1) * 512]
                    if ej % 2 == 0:
                        nc.scalar.copy(out=dst, in_=pt)
                    else:
                        nc.vector.tensor_copy(out=dst, in_=pt)

            for w_st, w_sb in ((wq_st, wq_sb), (wk_st, wk_sb)):
                nc.vector.tensor_copy(out=w_sb, in_=w_st)
            nc.vector.tensor_copy(out=wp_sb, in_=wp_st)

            # ---------------- v (all heads per s-block)
            nc.vector.memset(vt[:, :, :, 64:65], 1.0)
            for si in range(SB):
                pv = ppool.tile([P, E], F32, tag="pa", name="pv")
                for ej in range(EB):
                    nc.tensor.matmul(pv, lhsT=xT[ej][:, si * P:(si + 1) * P],
                                     rhs=wv_sb[:, ej], start=(ej == 0),
                                     stop=(ej == EB - 1))
                nc.vector.tensor_copy(out=vt[:, :, si, 0:64],
                                      in_=pv.rearrange("p (h d) -> p h d", h=H))

            # ---------------- qT / kT per group, interleaved with stripe-0
            # attention so exp starts early and the PE stays dense
            def emit_qkt(g):
                for w_sb, dst in ((wk_sb, kT[g]), (wq_sb, qT[g])):
                    for q0 in range(0, S, 1024):
                        pq = ppool.tile([P, 1024], F32, tag="pa", name="pq")
                        for ej in range(EB):
                            for c in range(q0, q0 + 1024, 512):
                                nc.tensor.matmul(
                                    pq[:, c - q0:c - q0 + 512],
                                    lhsT=w_sb[:, ej, 2 * g:2 * g + 2, :],
                                    rhs=xT[ej][:, c:c + 512],
                                    start=(ej == 0), stop=(ej == EB - 1))
                        nc.vector.tensor_copy(out=dst[:, q0:q0 + 1024], in_=pq)

            for g in range(G):
                emit_qkt(g)
                emit_att_head(0, 2 * g)
                emit_att_head(0, 2 * g + 1)

        emit_stripe_recip(0)
        for h in range(H):
            emit_att_head(1, h)
            emit_norm(0, h)
        emit_stripe_recip(1)
        for si in range(SW // P):
            emit_proj(si)
        for h in range(H):
            emit_norm(1, h)
        for si in range(SW // P, S // P):
            emit_proj(si)

    _hoist_matmul_waits(nc)
    return nc


def _hoist_matmul_waits(nc):
    """Several TRN2 ISA structs (notably the fp32 self-loading Matmult's LW
    struct) accept only one sync-wait slot; walrus dies with "Too many sync
    wait commands" otherwise. Hoist every wait of a multi-wait instruction
    onto same-engine NoOps inserted right before it (same engine queue =>
    identical ordering semantics)."""
    nid = [0]
    for fn in nc.m.functions:
        for blk in fn.blocks:
            insts = blk.instructions
            out = []
            for inst in insts:
                si = inst.sync_info
                if (inst.engine != mybir.EngineType.Unassigned and si is not None
                        and len(si.on_wait) >= 2 and inst.opcode != "NoOp"):
                    for w in si.on_wait:
                        nid[0] += 1
                        nop = mybir.InstNoOp(name=f"I-mmwait-{nid[0]}",
                                             ins=[], outs=[])
                        nop.engine = inst.engine
                        nop.sync_info = mybir.SyncInfo(on_wait=[w], on_update=[])
                        nc.inst_map[nop.name] = nop
                        out.append(nop)
                    inst.sync_info = mybir.SyncInfo(on_wait=[],
                                                    on_update=list(si.on_update))
                out.append(inst)
            if len(out) != len(insts):
                insts[:] = out


_nc_cache = {}


def _get_nc(S=S_FULL):
    if S not in _nc_cache:
        _nc_cache[S] = build_attention_nc(S)
    return _nc_cache[S]


def kernel(x, Wq, Wk, Wv, Wp, bp, _trace=False):
    nc = _get_nc(x.shape[1])
    n = x.shape[0]
    wq = np.ascontiguousarray(Wq, np.float32)
    wk = np.ascontiguousarray(Wk, np.float32)
    wv = np.ascontiguousarray(Wv, np.float32)
    wp = np.ascontiguousarray(Wp, np.float32)
    bpc = np.ascontiguousarray(bp, np.float32)
    in_maps = [
        {"x": np.ascontiguousarray(x[b], np.float32),
         "Wq": wq, "Wk": wk, "Wv": wv, "Wp": wp, "bp": bpc}
        for b in range(n)
    ]
    res = bass_utils.run_bass_kernel_spmd(
        nc, in_maps, core_ids=list(range(n)), trace=_trace)
    out = np.stack([r["out"] for r in res.results], axis=0)
    if _trace:
        return out, res
    return out


# revision 33
# speedup vs baseline: 1.5662x; 1.1191x over previous
"""Multi-head causal attention (B=8, S=2048, E=512, H=8, D=64) on 8 trn2 cores.

Strategy: pure data parallelism over the batch dimension (B == n_cores == 8).
Each NeuronCore computes the full attention for one batch element; no
collectives are needed.

v2 (vs baseline): attention matmul operands in bf16 (1 cyc/row on PE at any
width; fp32r pays 4x under 256 cols), x loaded in 4 batched DMAs instead of
16 (DMA-issue on the sync queue costs ~626ns each), qkT emission interleaved
with stripe-0 attention so ScalarE's exp stream starts early and the PE never
idles long enough for the HAM clock gate to re-throttle to 1.2 GHz, diagonal
causal mask applied as a DVE multiply with a precomputed 0/1 bf16 mask
(instead of a PE mask-matmul), softmax normalization via
reciprocal_approx_fast + PE ones-broadcast (instead of full-precision DVE
reciprocal at ~6.5us and a DRAM round-trip), PSUM->SBUF staging copy and
proj bias-add on GpSimd (idle otherwise), and the exp activation table
pre-warmed at kernel start.

Per core:
  1. Load x [S,E] (4 batched DMAs), transpose on PE -> xT bf16 [E,S].
  2. QKV: qT/kT per 2-head group g from wq/wk (bf16), v stored interleaved
     as vt[p=sk, h, j, 0:64] bf16 with vt[..., 64] = 1.0 so the AV matmul's
     stationary [128, 65] also produces the softmax denominator (row 64).
  3. Per sq-stripe (1024) and head: scoresT[sk,sq] = kT_j^T @ qT (K=64,
     bf16), exp via ScalarE (scale=1/8, no max subtraction needed:
     |scores/8| <~ 2) -> sb bf16, diag block masked by multiplying with a
     strictly-lower-zero mask on DVE, outT_aug[65,sq] += vt_j^T @ attT_j.
     Software-pipelined two deep so the PE does not stall on exp.
  4. Normalize: pst = po staged to SBUF (GpSimd, frees the PSUM bank),
     r = 1/l via reciprocal_approx_fast (DVE), broadcast r across 64
     partitions with a PE ones-matmul, attoutT = pst * r_bcast (DVE, bf16).
  5. Output projection out[s,e] = attoutT^T @ Wp (bf16) + bp (GpSimd add)
     -> HBM, interleaved with the next stripe's attention.

A single PSUM pool with two tags (4+4 banks) is used for the whole kernel.
Post-scheduling, excess semaphore waits are hoisted onto same-engine NoOps
(several ISA structs accept only one wait slot; walrus rejects multi-wait
instructions).
"""

import numpy as np
from contextlib import ExitStack

import concourse.bass as bass
import concourse.mybir as mybir
from concourse.tile import TileContext
from concourse.masks import make_identity
from concourse import bass_utils

F32 = mybir.dt.float32
F32R = mybir.dt.float32r
BF16 = mybir.dt.bfloat16
B, S_FULL, E, H, D = 8, 2048, 512, 8, 64
P = 128
G = H // 2      # 2-head groups
EB = E // P     # e blocks
EXP = mybir.ActivationFunctionType.Exp


def build_attention_nc(S=S_FULL):
    SB = S // P                 # s blocks
    SW = min(1024, S)           # stripe width (sq columns)
    NS = S // SW                # number of stripes
    nc = bass.Bass(trn_type="TRN2")

    x_d = nc.dram_tensor("x", [S, E], F32, kind="ExternalInput").ap()
    wq_d = nc.dram_tensor("Wq", [H, E, D], F32, kind="ExternalInput").ap()
    wk_d = nc.dram_tensor("Wk", [H, E, D], F32, kind="ExternalInput").ap()
    wv_d = nc.dram_tensor("Wv", [H, E, D], F32, kind="ExternalInput").ap()
    wp_d = nc.dram_tensor("Wp", [E, E], F32, kind="ExternalInput").ap()
    bp_d = nc.dram_tensor("bp", [E], F32, kind="ExternalInput").ap()
    out_d = nc.dram_tensor("out", [S, E], F32, kind="ExternalOutput").ap()
    # scratch for the batched softmax-denominator reciprocal: l rows go out
    # in row layout, come back partition-spread [128, H*SW/128], and return
    # reciprocal'd in row layout for the PE broadcast matmul.
    lscr = nc.dram_tensor("lscr", [NS * H * SW], BF16, kind="Internal").ap()
    rscr = nc.dram_tensor("rscr", [NS * H * SW], BF16, kind="Internal").ap()

    with TileContext(nc) as tc, ExitStack() as top:
        const = top.enter_context(tc.tile_pool(name="const", bufs=1))
        warm = const.tile([1, 2], F32, tag="warm")
        nc.vector.memset(warm, 0.0)
        # pre-warm the exp table set (~2.7us) off the critical path
        nc.scalar.activation(out=warm[0:1, 1:2], in_=warm[0:1, 0:1], func=EXP)
        ident = const.tile([P, P], F32, tag="ident")
        make_identity(nc, ident)
        ones = const.tile([1, D], BF16, tag="ones")
        nc.vector.memset(ones, 1.0)
        bpb = const.tile([P, E], F32, tag="bpb")
        wp_sb = const.tile([P, G, E], BF16, tag="wp")

        out_pool = top.enter_context(tc.tile_pool(name="outsb", bufs=2))
        qkv = top.enter_context(tc.tile_pool(name="qkv", bufs=1))
        qT = [qkv.tile([P, S], BF16, tag=f"qT{g}", name=f"qT{g}") for g in range(G)]
        kT = [qkv.tile([P, S], BF16, tag=f"kT{g}", name=f"kT{g}") for g in range(G)]
        vt = qkv.tile([P, H, SB, 65], BF16, tag="vt")
        attp = top.enter_context(tc.tile_pool(name="attsb", bufs=4))

        # single PSUM pool: tag "pa" = working (2 banks x3, lets scores run
        # 3 j-blocks ahead of exp), tag "po" = attention output accumulator
        # (2 banks x1; freed by the post-AV copies before the next head's
        # first AV needs it)
        ppool = top.enter_context(tc.tile_pool(name="ppool", bufs=3, space="PSUM"))

        attout = top.enter_context(tc.tile_pool(name="attout", bufs=1))
        attoutT = [attout.tile([P, G, SW], BF16, tag=f"attoutT{t}",
                               name=f"attoutT{t}") for t in range(NS)]
        rpool = top.enter_context(tc.tile_pool(name="rp", bufs=2))
        # r rows are DMA'd back into the same strip the l rows left from
        # (WAR tracked by the tile layer; saves 32KB/partition of SBUF)
        lrow = [attout.tile([1, H * SW], BF16, tag=f"lrow{t}", name=f"lrow{t}")
                for t in range(NS)]
        rrow = lrow

        # ---------------- attention (per stripe x head) + interleaved proj
        def emit_av(po, h, pend, lo, hi):
            sb, j = pend
            jlo = max(lo, j * P)
            for b in range(lo, hi, 512):
                clo, chi = max(jlo, b), b + 512
                if clo >= chi:
                    continue
                nc.tensor.matmul(po[:, clo - lo:chi - lo],
                                 lhsT=vt[:, h, j, :],
                                 rhs=sb[:, clo - lo:chi - lo],
                                 start=(j == 0), stop=(j == chi // P - 1))

        def emit_proj(si):
            tt, col = si * P // SW, (si * P) % SW
            pp = ppool.tile([P, E], F32, tag="pa", name="pp")
            for g in range(G):
                nc.tensor.matmul(pp, lhsT=attoutT[tt][:, g, col:col + P],
                                 rhs=wp_sb[:, g, :], start=(g == 0),
                                 stop=(g == G - 1))
            ob = out_pool.tile([P, E], F32, tag="ob", name="ob")
            nc.vector.tensor_add(out=ob, in0=pp, in1=bpb)
            nc.sync.dma_start(out=out_d[si * P:(si + 1) * P, :], in_=ob)

        def emit_att_head(t, h):
            lo, hi = t * SW, (t + 1) * SW
            jmax = hi // P
            g, hh = h // 2, (h % 2) * D
            po = ppool.tile([65, SW], F32, tag="po", name="po", bufs=1)
            pending = []
            for j in range(jmax):
                jlo = max(lo, j * P)
                ps = ppool.tile([P, SW], F32, tag="pa", name="ps")
                for b in range(lo, hi, 512):
                    clo, chi = max(jlo, b), b + 512
                    if clo >= chi:
                        continue
                    nc.tensor.matmul(ps[:, clo - lo:chi - lo],
                                     lhsT=kT[g][hh:hh + D, j * P:(j + 1) * P],
                                     rhs=qT[g][hh:hh + D, clo:chi],
                                     start=True, stop=True)
                if len(pending) >= 3:
                    emit_av(po, h, pending.pop(0), lo, hi)
                sb = attp.tile([P, SW], BF16, tag="attsb", name="sb")
                nc.scalar.activation(out=sb[:, jlo - lo:], in_=ps[:, jlo - lo:],
                                     func=EXP, scale=float(1.0 / np.sqrt(D)))
                if j * P >= lo:
                    # zero the strictly-lower triangle of the diagonal block
                    # (on GpSimd: keeps it out of the DVE FIFO, which the
                    # AV matmuls depend on via the norm-chain ops)
                    dcol = j * P - lo
                    nc.gpsimd.affine_select(
                        out=sb[:, dcol:dcol + P], in_=sb[:, dcol:dcol + P],
                        compare_op=mybir.AluOpType.is_ge, fill=0.0,
                        base=0, pattern=[[1, P]], channel_multiplier=-1)
                pending.append((sb, j))
            for pend in pending:
                emit_av(po, h, pend, lo, hi)
            # store the UNNORMALIZED attention output (scaled in place once
            # the batched reciprocal row returns) and stage the denominator
            # row l into this stripe's row-layout staging strip (f32r copy
            # rounds it for the f32r broadcast matmul).
            nc.vector.tensor_copy(out=attoutT[t][hh:hh + D, g, :],
                                  in_=po[0:D, :])
            nc.vector.tensor_copy(out=lrow[t][0:1, h * SW:(h + 1) * SW],
                                  in_=po[D:D + 1, :])

        def emit_stripe_recip(t):
            # all H denominator rows of stripe t -> DRAM -> partition-spread
            # [128, H*SW/128] -> one cheap DVE reciprocal -> DRAM -> row
            # layout for the per-head PE broadcast.
            n = H * SW
            off = t * n
            nc.sync.dma_start(
                out=bass.AP(tensor=lscr.tensor, offset=lscr.offset + off,
                            ap=[[0, 1], [1, n]]),
                in_=lrow[t])
            lsp = rpool.tile([P, n // P], BF16, tag="lsp", name="lsp")
            nc.sync.dma_start(
                out=lsp,
                in_=bass.AP(tensor=lscr.tensor, offset=lscr.offset + off,
                            ap=[[n // P, P], [1, n // P]]))
            rsp = rpool.tile([P, n // P], BF16, tag="rsp", name="rsp")
            with nc.allow_low_precision("softmax denom reciprocal; rel-err "
                                        "budget 2e-2 >> bf16 eps"):
                nc.vector.reciprocal(out=rsp, in_=lsp)
            nc.sync.dma_start(
                out=bass.AP(tensor=rscr.tensor, offset=rscr.offset + off,
                            ap=[[n // P, P], [1, n // P]]),
                in_=rsp)
            nc.sync.dma_start(
                out=rrow[t],
                in_=bass.AP(tensor=rscr.tensor, offset=rscr.offset + off,
                            ap=[[0, 1], [1, n]]))

        def emit_norm(t, h):
            # attoutT[t] *= bcast(1/l) in place
            g, hh = h // 2, (h % 2) * D
            pbc = ppool.tile([D, SW], F32, tag="pa", name="pbc")
            for c in range(0, SW, 512):
                nc.tensor.matmul(
                    pbc[:, c:c + 512], lhsT=ones,
                    rhs=rrow[t][0:1, h * SW + c:h * SW + c + 512],
                    start=True, stop=True)
            sl = attoutT[t][hh:hh + D, g, :]
            nc.vector.tensor_mul(out=sl, in0=sl, in1=pbc)

        with ExitStack() as ph2:
            xin = ph2.enter_context(tc.tile_pool(name="xin", bufs=2))
            xt_pool = ph2.enter_context(tc.tile_pool(name="xt", bufs=1))
            wpool = ph2.enter_context(tc.tile_pool(name="wqkv", bufs=1))

            xT = [xt_pool.tile([P, S], BF16, tag=f"xT{e}", name=f"xT{e}")
                  for e in range(EB)]

            # ---------------- batched input DMAs, x interleaved with weights
            wq_r = wq_d.rearrange("h e d -> e h d")
            wk_r = wk_d.rearrange("h e d -> e h d")
            wv_r = wv_d.rearrange("h e d -> e h d")
            wq_st = wpool.tile([P, EB, H, D], F32, tag="wqs", name="wqs")
            wk_st = wpool.tile([P, EB, H, D], F32, tag="wks", name="wks")
            wv_st = wpool.tile([P, EB, H, D], F32, tag="wvs", name="wvs")
            wp_st = wpool.tile([P, G, E], F32, tag="wps", name="wps")
            # x in 8 half-MB chunks alternating the Sync/Scalar DMA queues
            # (two DGEs program in parallel; first chunk lands fast so the
            # PE transposes start early). wv is loaded first among the
            # weights (the v matmuls are the first weight consumers).
            xs4 = [xin.tile([P, 4, E], F32, tag="xs", name=f"xs{k4}")
                   for k4 in range(SB // 4)]
            w_dmas = []
            for w_r, w_st in ((wv_r, wv_st), (wq_r, wq_st), (wk_r, wk_st)):
                for ej in range(EB):
                    w_dmas.append((w_st[:, ej], w_r[ej * P:(ej + 1) * P, :, :]))
            for g in range(G):
                w_dmas.append((wp_st[:, g, :], wp_d[g * P:(g + 1) * P, :]))
            for k8 in range(SB // 2):
                k4, i2 = k8 // 2, (k8 % 2) * 2
                eng = nc.sync if k8 % 2 == 0 else nc.scalar
                eng.dma_start(
                    out=xs4[k4][:, i2:i2 + 2, :],
                    in_=x_d[k8 * 256:(k8 + 1) * 256, :].rearrange(
                        "(i p) e -> p i e", p=P))
            for wi, (dst, src) in enumerate(w_dmas):
                eng = nc.sync if wi % 2 == 0 else nc.scalar
                eng.dma_start(out=dst, in_=src)
            nc.sync.dma_start(
                out=bpb,
                in_=bass.AP(tensor=bp_d.tensor, offset=bp_d.offset,
                            ap=[[0, P]] + list(bp_d.ap)))

            # ---------------- weight casts fp32 -> bf16 (wv first: it lands
            # first and the v matmuls are its only consumer; wq/wk/wp casts
            # go after the xT copies so they don't block the DVE FIFO)
            wq_sb = wpool.tile([P, EB, H, D], BF16, tag="wq")
            wk_sb = wpool.tile([P, EB, H, D], BF16, tag="wk")
            wv_sb = wpool.tile([P, EB, H, D], BF16, tag="wv")
            nc.vector.tensor_copy(out=wv_sb, in_=wv_st)

            # ---------------- x PE-transpose -> xT bf16 (copies split
            # between ScalarE and DVE)
            for k4 in range(SB // 4):
                for ej in range(EB):
                    pt = ppool.tile([P, 512], F32, tag="pa", name="pt")
                    for i in range(4):
                        nc.tensor.transpose(pt[:, i * P:(i + 1) * P],
                                            xs4[k4][:, i, ej * P:(ej + 1) * P],
                                            ident)
                    dst = xT[ej][:, k4 * 512:(k4 + 1) * 512]
                    if ej % 2 == 0:
                        nc.scalar.copy(out=dst, in_=pt)
                    else:
                        nc.vector.tensor_copy(out=dst, in_=pt)

            for w_st, w_sb in ((wq_st, wq_sb), (wk_st, wk_sb)):
                nc.vector.tensor_copy(out=w_sb, in_=w_st)
            nc.vector.tensor_copy(out=wp_sb, in_=wp_st)

            # ---------------- v (all heads per s-block)
            nc.vector.memset(vt[:, :, :, 64:65], 1.0)
            for si in range(SB):
                pv = ppool.tile([P, E], F32, tag="pa", name="pv")
                for ej in range(EB):
                    nc.tensor.matmul(pv, lhsT=xT[ej][:, si * P:(si + 1) * P],
                                     rhs=wv_sb[:, ej], start=(ej == 0),
                                     stop=(ej == EB - 1))
                nc.vector.tensor_copy(out=vt[:, :, si, 0:64],
                                      in_=pv.rearrange("p (h d) -> p h d", h=H))

            # ---------------- qT / kT per group, interleaved with stripe-0
            # attention so exp starts early and the PE stays dense
            def emit_qkt(g):
                for w_sb, dst in ((wk_sb, kT[g]), (wq_sb, qT[g])):
                    for q0 in range(0, S, 1024):
                        pq = ppool.tile([P, 1024], F32, tag="pa", name="pq")
                        for ej in range(EB):
                            for c in range(q0, q0 + 1024, 512):
                                nc.tensor.matmul(
                                    pq[:, c - q0:c - q0 + 512],
                                    lhsT=w_sb[:, ej, 2 * g:2 * g + 2, :],
                                    rhs=xT[ej][:, c:c + 512],
                                    start=(ej == 0), stop=(ej == EB - 1))
                        nc.vector.tensor_copy(out=dst[:, q0:q0 + 1024], in_=pq)

            for g in range(G):
                emit_qkt(g)
                emit_att_head(0, 2 * g)
                emit_att_head(0, 2 * g + 1)

        # stripe-0 norms and projs are spread between the stripe-1 heads:
        # stripe-1 attention is ScalarE(exp)-bound, and this PE-side work
        # fills the j-pipeline stalls so the HAM clock gate stays at 8/8.
        # The norms start only after two heads (~25us) so the in-order PE
        # queue never waits on the batched-reciprocal DMA round-trip.
        emit_stripe_recip(0)
        emit_att_head(1, 0)
        emit_att_head(1, 1)
        for h in range(H):
            emit_norm(0, h)
        stripe0_projs = {2: [0], 3: [1], 4: [2], 5: [3], 6: [4, 5], 7: [6, 7]}
        for h in range(2, H):
            emit_att_head(1, h)
            for si in stripe0_projs[h]:
                emit_proj(si)
        emit_stripe_recip(1)
        for h in range(H):
            emit_norm(1, h)
        for si in range(SW // P, S // P):
            emit_proj(si)

    _hoist_matmul_waits(nc)
    return nc


def _hoist_matmul_waits(nc):
    """Several TRN2 ISA structs (notably the fp32 self-loading Matmult's LW
    struct) accept only one sync-wait slot; walrus dies with "Too many sync
    wait commands" otherwise. Hoist every wait of a multi-wait instruction
    onto same-engine NoOps inserted right before it (same engine queue =>
    identical ordering semantics)."""
    nid = [0]
    for fn in nc.m.functions:
        for blk in fn.blocks:
            insts = blk.instructions
            out = []
            for inst in insts:
                si = inst.sync_info
                if (inst.engine != mybir.EngineType.Unassigned and si is not None
                        and len(si.on_wait) >= 2 and inst.opcode != "NoOp"):
                    for w in si.on_wait:
                        nid[0] += 1
                        nop = mybir.InstNoOp(name=f"I-mmwait-{nid[0]}",
                                             ins=[], outs=[])
                        nop.engine = inst.engine
                        nop.sync_info = mybir.SyncInfo(on_wait=[w], on_update=[])
                        nc.inst_map[nop.name] = nop
                        out.append(nop)
                    inst.sync_info = mybir.SyncInfo(on_wait=[],
                                                    on_update=list(si.on_update))
                out.append(inst)
            if len(out) != len(insts):
                insts[:] = out


_nc_cache = {}


def _get_nc(S=S_FULL):
    if S not in _nc_cache:
        _nc_cache[S] = build_attention_nc(S)
    return _nc_cache[S]


def kernel(x, Wq, Wk, Wv, Wp, bp, _trace=False):
    nc = _get_nc(x.shape[1])
    n = x.shape[0]
    wq = np.ascontiguousarray(Wq, np.float32)
    wk = np.ascontiguousarray(Wk, np.float32)
    wv = np.ascontiguousarray(Wv, np.float32)
    wp = np.ascontiguousarray(Wp, np.float32)
    bpc = np.ascontiguousarray(bp, np.float32)
    in_maps = [
        {"x": np.ascontiguousarray(x[b], np.float32),
         "Wq": wq, "Wk": wk, "Wv": wv, "Wp": wp, "bp": bpc}
        for b in range(n)
    ]
    res = bass_utils.run_bass_kernel_spmd(
        nc, in_maps, core_ids=list(range(n)), trace=_trace)
    out = np.stack([r["out"] for r in res.results], axis=0)
    if _trace:
        return out, res
    return out
